# revision 1
# baseline (speedup 1.0000x reference)
"""Trainium2 Bass kernel for nn_BlockDecomposition (relational GNN message passing).

out[n] = sum_r sum_{e: type=r, tgt=n} w_e * (x[src_e] @ BD(blocks[r]))

Sharding: relation r -> core r (R == n_cores == 8). Each core:
  phase 1: y = x @ BD(blocks[r])          (391 matmuls, y in blocked DRAM layout)
  phase 2: windowed gather/aggregate:
           - edges sorted by target into 391 windows of 128 targets
           - per window, edges split into two source-half streams (int16 gather
             index limit), each padded to a fixed chunk count (C_LO/C_HI chosen
             from the data at build time, same for all cores -> one SPMD program)
           - dma_gather batches of 4096 rows; fused one-hot (iota==tgt)*w on DVE;
             PSUM-accumulated matmul per window; staged 2KB blocked output writes
Host: unblock + sum the 8 per-relation partials.
"""
import os
import numpy as np

N_NODES = 50000
N_PAD = 50048            # 391 * 128
NWIN = N_PAD // 128      # 391
D = 64
R = 8
P = 128
SPLIT = 32768            # source-half split (int16 index range)
BATCH_CH = 16            # chunks per dma_gather (2048 indices)

_cache = {}


def _build_program(c_lo, c_hi):
    import concourse.bacc as bacc
    import concourse.bass as bass
    import concourse.tile as tile
    import concourse.mybir as mybir

    cpw = c_lo + c_hi                 # chunks per window
    nch_lo = NWIN * c_lo
    nch_hi = NWIN * c_hi
    nch = NWIN * cpw

    nc = bacc.Bacc("TRN2", target_bir_lowering=False, debug=False, num_devices=8,
                   num_swdge_queues=4)

    xT_d = nc.dram_tensor("xT", [D, N_PAD], mybir.dt.float32, kind="ExternalInput")
    wbd_d = nc.dram_tensor("wbd", [D, D], mybir.dt.float32, kind="ExternalInput")
    il_d = nc.dram_tensor("il", [P, nch_lo * 8], mybir.dt.int16, kind="ExternalInput")
    ih_d = nc.dram_tensor("ih", [P, nch_hi * 8], mybir.dt.int16, kind="ExternalInput")
    wgt_d = nc.dram_tensor("wgt", [P, nch], mybir.dt.float32, kind="ExternalInput")
    tgt_d = nc.dram_tensor("tgt", [P, nch], mybir.dt.float32, kind="ExternalInput")
    iota_d = nc.dram_tensor("iota", [P, P], mybir.dt.float32, kind="ExternalInput")
    out_d = nc.dram_tensor("out", [N_PAD, D], mybir.dt.float32, kind="ExternalOutput")

    XCOLS = 6272          # n-tile columns per xT SBUF tile (49 windows)
    n_xtiles = (N_PAD + XCOLS - 1) // XCOLS

    with tile.TileContext(nc) as tc:
        with (
            tc.tile_pool(name="consts", bufs=1) as consts,
            tc.tile_pool(name="ydram", bufs=1, space="DRAM") as ydram,
            tc.tile_pool(name="xt", bufs=2) as xt_pool,
            tc.tile_pool(name="ystage", bufs=2) as ystage_pool,
            tc.tile_pool(name="p1psum", bufs=2, space="PSUM") as p1psum,
            tc.tile_pool(name="edges", bufs=1) as edges,
            tc.tile_pool(name="msgs", bufs=6) as msgs_pool,
            tc.tile_pool(name="oh", bufs=6) as oh_pool,
            tc.tile_pool(name="psum", bufs=6, space="PSUM") as psum_pool,
            tc.tile_pool(name="evict", bufs=2) as evict_pool,
        ):
            iota_f = consts.tile([P, P], mybir.dt.float32, tag="iota")
            nc.sync.dma_start(iota_f[:], iota_d[:])
            wbd_t = consts.tile([D, D], mybir.dt.float32, tag="wbd")
            nc.sync.dma_start(wbd_t[:], wbd_d[:])

            y_t = ydram.tile([N_PAD, D], mybir.dt.float32, tag="y")
            # blocked view: row (p*NWIN + w) holds node n = w*128 + p
            y_bl = y_t[:].rearrange("(p w) e -> p (w e)", p=P)

            # ---- phase 1: y = x @ W_bd ----
            st = None
            for xt_i in range(n_xtiles):
                col0 = xt_i * XCOLS
                cols = min(XCOLS, N_PAD - col0)
                nwt = cols // P
                xt = xt_pool.tile([D, XCOLS], mybir.dt.float32, tag="xt")
                nc.sync.dma_start(xt[:, :cols], xT_d[:, col0:col0 + cols])
                for ti in range(nwt):
                    t_glob = xt_i * (XCOLS // P) + ti
                    ps = p1psum.tile([P, D], mybir.dt.float32, space="PSUM", tag="p1")
                    nc.tensor.matmul(
                        out=ps[:], lhsT=xt[:, ti * P:(ti + 1) * P], rhs=wbd_t[:],
                        start=True, stop=True)
                    si = t_glob % 8
                    if si == 0:
                        st = ystage_pool.tile([P, 8 * D], mybir.dt.float32, tag="yst")
                    nc.scalar.copy(st[:, si * D:(si + 1) * D], ps[:])
                    if si == 7 or t_glob == NWIN - 1:
                        t0 = t_glob - si
                        nc.sync.dma_start(
                            y_bl[:, t0 * D:(t_glob + 1) * D], st[:, :(si + 1) * D])

            # ---- edge streams ----
            il_t = edges.tile([P, nch_lo * 8], mybir.dt.int16, tag="il")
            ih_t = edges.tile([P, nch_hi * 8], mybir.dt.int16, tag="ih")
            wgt_t = edges.tile([P, nch], mybir.dt.float32, tag="wgt")
            tgt_t = edges.tile([P, nch], mybir.dt.float32, tag="tgt")
            nc.sync.dma_start(il_t[:], il_d[:])
            nc.sync.dma_start(ih_t[:], ih_d[:])
            nc.sync.dma_start(wgt_t[:], wgt_d[:])
            nc.sync.dma_start(tgt_t[:], tgt_d[:])

            # ---- phase 2: gather + one-hot matmul aggregation ----
            y_lo = y_t[0:SPLIT, :]
            y_hi = y_t[SPLIT:N_PAD, :]

            qrr = [0]

            def emit_gather(b, nch_s, idx_tile, src_ap, tag):
                ch = min(BATCH_CH, nch_s - b * BATCH_CH)
                ni = ch * P
                mt = msgs_pool.tile([P, BATCH_CH * D], mybir.dt.float32, tag=tag)
                nc.gpsimd.dma_gather(
                    out_ap=mt[:, :ch * D].rearrange("p (c e) -> p c e", e=D),
                    in_ap=src_ap,
                    idxs_ap=idx_tile[:, b * BATCH_CH * 8:b * BATCH_CH * 8 + ch * 8],
                    num_idxs=ni, num_idxs_reg=ni, elem_size=D,
                    single_packet=False, queue_num=qrr[0] % 4)
                qrr[0] += 1
                return mt

            # emit gathers interleaved in window-consumption order: batch b of a
            # stream with chunks-per-window c covers windows [b*BATCH_CH/c, ...)
            nb_lo = (nch_lo + BATCH_CH - 1) // BATCH_CH
            nb_hi = (nch_hi + BATCH_CH - 1) // BATCH_CH if c_hi else 0
            ev = []
            for b in range(nb_lo):
                ev.append((b * BATCH_CH // c_lo, 0, b))
            for b in range(nb_hi):
                ev.append((b * BATCH_CH // c_hi, 1, b))
            ev.sort()
            lo_tiles, hi_tiles = {}, {}
            for _, s, b in ev:
                if s == 0:
                    lo_tiles[b] = emit_gather(b, nch_lo, il_t, y_lo, "mlo")
                else:
                    hi_tiles[b] = emit_gather(b, nch_hi, ih_t, y_hi, "mhi")

            out_bl = out_d[:].rearrange("(p w) e -> p (w e)", p=P)
            stg = None
            for w in range(NWIN):
                ps = psum_pool.tile([P, D], mybir.dt.float32, space="PSUM", tag="agg")
                for c in range(cpw):
                    j = w * cpw + c
                    if c < c_lo:
                        js = w * c_lo + c
                        mt = lo_tiles[js // BATCH_CH]
                    else:
                        js = w * c_hi + (c - c_lo)
                        mt = hi_tiles[js // BATCH_CH]
                    jl = js % BATCH_CH
                    oh = oh_pool.tile([P, P], mybir.dt.float32, tag="oh")
                    nc.vector.tensor_scalar(
                        out=oh[:], in0=iota_f[:],
                        scalar1=tgt_t[:, j:j + 1], scalar2=wgt_t[:, j:j + 1],
                        op0=mybir.AluOpType.is_equal, op1=mybir.AluOpType.mult)
                    nc.tensor.matmul(
                        out=ps[:], lhsT=oh[:], rhs=mt[:, jl * D:(jl + 1) * D],
                        start=(c == 0), stop=(c == cpw - 1))
                si = w % 8
                if si == 0:
                    stg = evict_pool.tile([P, 8 * D], mybir.dt.float32, tag="ostg")
                nc.vector.tensor_copy(stg[:, si * D:(si + 1) * D], ps[:])
                if si == 7 or w == NWIN - 1:
                    w0 = w - si
                    nc.sync.dma_start(
                        out_bl[:, w0 * D:(w + 1) * D], stg[:, :(si + 1) * D])

    nc.compile()
    return nc


def _prep_core(src, tgt, wgt, c_lo, c_hi):
    """Build per-core edge streams. Returns il, ih, wgt_arr, tgt_arr."""
    cpw = c_lo + c_hi
    nch_lo = NWIN * c_lo
    nch_hi = NWIN * c_hi
    nch = NWIN * cpw

    # blocked source index
    bidx = (src % P).astype(np.int64) * NWIN + src // P
    win = tgt // P
    toff = (tgt % P).astype(np.float32)

    il = np.zeros(nch_lo * P, np.int16)
    ih = np.zeros(nch_hi * P, np.int16)
    wgt_arr = np.zeros((P, nch), np.float32)
    tgt_arr = np.zeros((P, nch), np.float32)

    order = np.argsort(win, kind="stable")
    win_s = win[order]
    bidx_s = bidx[order]
    wgt_s = wgt[order]
    toff_s = toff[order]
    lo_mask = bidx_s < SPLIT

    starts = np.searchsorted(win_s, np.arange(NWIN + 1))
    for w in range(NWIN):
        s0, s1 = starts[w], starts[w + 1]
        lm = lo_mask[s0:s1]
        for is_lo, c_n, stream, base in ((True, c_lo, il, 0), (False, c_hi, ih, SPLIT)):
            sel = lm if is_lo else ~lm
            n = int(sel.sum())
            cap = c_n * P
            assert n <= cap, (w, n, cap)
            if is_lo:
                pos0 = w * c_lo * P
            else:
                pos0 = w * c_hi * P
            stream[pos0:pos0 + n] = (bidx_s[s0:s1][sel] - base).astype(np.int16)
            # slot -> global chunk j and lane
            slots = np.arange(cap)
            cw = slots // P          # chunk within this window's stream
            lane = slots % P
            j = w * cpw + (cw if is_lo else c_lo + cw)
            wcol = np.zeros(cap, np.float32)
            tcol = np.zeros(cap, np.float32)
            wcol[:n] = wgt_s[s0:s1][sel]
            tcol[:n] = toff_s[s0:s1][sel]
            wgt_arr[lane, j] = wcol
            tgt_arr[lane, j] = tcol

    def wrap(stream, nch_s):
        # slot (chunk js, lane p) at stream[js*128+p] -> wrapped (128, nch_s*8)
        out = np.zeros((P, nch_s * 8), np.int16)
        nb = (nch_s + BATCH_CH - 1) // BATCH_CH
        for b in range(nb):
            ch = min(BATCH_CH, nch_s - b * BATCH_CH)
            seg = stream[b * BATCH_CH * P: b * BATCH_CH * P + ch * P]
            w16 = seg.reshape(ch * 8, 16).T           # (16, ch*8)
            out[:, b * BATCH_CH * 8: b * BATCH_CH * 8 + ch * 8] = np.tile(w16, (8, 1))
        return out

    return wrap(il, nch_lo), wrap(ih, nch_hi), wgt_arr, tgt_arr


def kernel(x, blocks, edge_weights, source, target, edge_type):
    from concourse.bass_utils import run_bass_kernel_spmd

    x = np.asarray(x, np.float32)
    blocks = np.asarray(blocks, np.float32)
    edge_weights = np.asarray(edge_weights, np.float32)
    source = np.asarray(source, np.int64)
    target = np.asarray(target, np.int64)
    edge_type = np.asarray(edge_type, np.int64)

    n, d = x.shape
    assert n == N_NODES and d == D

    xp = np.zeros((N_PAD, D), np.float32)
    xp[:n] = x
    xT = np.ascontiguousarray(xp.T)

    iota = np.broadcast_to(np.arange(P, dtype=np.float32), (P, P)).copy()

    # per-core edge sets + stream capacity
    per_core = []
    c_lo = c_hi = 1
    for r in range(R):
        m = edge_type == r
        src, tgt, wgt = source[m], target[m], edge_weights[m]
        bidx = (src % P) * NWIN + src // P
        win = tgt // P
        lo = bidx < SPLIT
        cnt_lo = np.bincount(win[lo], minlength=NWIN)
        cnt_hi = np.bincount(win[~lo], minlength=NWIN)
        c_lo = max(c_lo, int(-(-cnt_lo.max() // P)))
        c_hi = max(c_hi, int(-(-cnt_hi.max() // P)))
        per_core.append((src, tgt, wgt))

    key = (c_lo, c_hi)
    if key not in _cache:
        _cache[key] = _build_program(c_lo, c_hi)
    nc = _cache[key]

    in_maps = []
    for r in range(R):
        src, tgt, wgt = per_core[r]
        il, ih, wgt_arr, tgt_arr = _prep_core(src, tgt, wgt, c_lo, c_hi)
        wbd = np.zeros((D, D), np.float32)
        bs = D // blocks.shape[1]
        for b in range(blocks.shape[1]):
            wbd[b * bs:(b + 1) * bs, b * bs:(b + 1) * bs] = blocks[r, b]
        in_maps.append({
            "xT": xT, "wbd": wbd, "il": il, "ih": ih,
            "wgt": wgt_arr, "tgt": tgt_arr, "iota": iota,
        })

    res = run_bass_kernel_spmd(nc, in_maps, core_ids=list(range(R)))

    out = np.zeros((N_PAD, D), np.float32)
    for r in range(R):
        bl = res.results[r]["out"].reshape(P, NWIN, D)
        out += bl.transpose(1, 0, 2).reshape(N_PAD, D)
    return out[:N_NODES]



# revision 5
# speedup vs baseline: 1.8398x; 1.8398x over previous
"""Trainium2 Bass kernel for nn_BlockDecomposition (relational GNN message passing).

out[n] = sum_r sum_{e: type=r, tgt=n} w_e * (x[src_e] @ BD(blocks[r]))

Sharding: targets -> cores. Core c owns 49 windows of 128 targets
(392 windows total, window 391 is padding). For each (window, relation)
bucket the core:
  - gathers x[src] rows (bf16, duplicated to 256B tokens) via dma_gather
    straight from DRAM (no pre-transform pass),
  - builds a weighted one-hot [e=128, t=128] on DVE (bf16, is_equal*mult),
  - aggregates aggT[d, t] += msgs^T @ oh in PSUM (bf16 matmuls),
  - applies the relation's block-diagonal W via a second matmul,
    accumulating all 8 relations of the window into one PSUM tile,
  - evicts out2T[d, t] f32 through the Scalar engine + staged DMA.
Host: concatenate the 8 per-core [64, 6272] transposed slabs.

Edge streams are split into lo/hi source halves (int16 gather index limit)
padded to fixed per-bucket chunk counts so all 8 cores run one SPMD program.
"""
import numpy as np

N_NODES = 50000
N_PAD = 50048            # 391 * 128
P = 128
D = 64
R = 8
WPC = 49                 # windows per core (8*49=392 >= 391, last is padding)
NWIN_PC = WPC * R        # buckets per core (window-major, relation-minor)
SLAB = WPC * P           # 6272 targets per core
SPLIT = 32768            # source-half split (int16 gather index limit)
BATCH_CH = 32            # chunks per dma_gather (4096 indices)

_cache = {}


def _build_program(c_lo, c_hi):
    import concourse.bacc as bacc
    import concourse.tile as tile
    import concourse.mybir as mybir

    cpw = c_lo + c_hi                # chunks per bucket
    nb = NWIN_PC                     # buckets per core
    nch_lo = nb * c_lo
    nch_hi = nb * c_hi
    nch = nb * cpw

    nc = bacc.Bacc("TRN2", target_bir_lowering=False, debug=False, num_devices=8,
                   num_swdge_queues=4)

    xd_d = nc.dram_tensor("xd", [N_PAD, 2 * D], mybir.dt.bfloat16, kind="ExternalInput")
    il_d = nc.dram_tensor("il", [P, nch_lo * 8], mybir.dt.int16, kind="ExternalInput")
    ih_d = nc.dram_tensor("ih", [P, nch_hi * 8], mybir.dt.int16, kind="ExternalInput")
    wgt_d = nc.dram_tensor("wgt", [P, nch], mybir.dt.float32, kind="ExternalInput")
    tgt_d = nc.dram_tensor("tgt", [P, nch], mybir.dt.float32, kind="ExternalInput")
    iota_d = nc.dram_tensor("iota", [P, P], mybir.dt.bfloat16, kind="ExternalInput")
    wtab_d = nc.dram_tensor("wtab", [D, R * D], mybir.dt.bfloat16, kind="ExternalInput")
    out_d = nc.dram_tensor("outT", [D, WPC * P], mybir.dt.float32,
                           kind="ExternalOutput")  # [64, 6272]

    with tile.TileContext(nc) as tc:
        with (
            tc.tile_pool(name="consts", bufs=1) as consts,
            tc.tile_pool(name="edges", bufs=1) as edges,
            tc.tile_pool(name="msgs", bufs=8) as msgs_pool,
            tc.tile_pool(name="oh", bufs=8) as oh_pool,
            tc.tile_pool(name="agg", bufs=4, space="PSUM") as agg_pool,
            tc.tile_pool(name="absb", bufs=4) as absb_pool,
            tc.tile_pool(name="out2", bufs=2, space="PSUM") as out2_pool,
            tc.tile_pool(name="evict", bufs=2) as evict_pool,
        ):
            iota_t = consts.tile([P, P], mybir.dt.bfloat16, tag="iota")
            nc.sync.dma_start(iota_t[:], iota_d[:])
            wtab_t = consts.tile([D, R * D], mybir.dt.bfloat16, tag="wtab")
            nc.sync.dma_start(wtab_t[:], wtab_d[:])

            il_t = edges.tile([P, nch_lo * 8], mybir.dt.int16, tag="il")
            ih_t = edges.tile([P, nch_hi * 8], mybir.dt.int16, tag="ih")
            wgt_t = edges.tile([P, nch], mybir.dt.float32, tag="wgt")
            tgt_t = edges.tile([P, nch], mybir.dt.float32, tag="tgt")
            nc.sync.dma_start(il_t[:], il_d[:])
            nc.sync.dma_start(ih_t[:], ih_d[:])
            nc.sync.dma_start(wgt_t[:], wgt_d[:])
            nc.sync.dma_start(tgt_t[:], tgt_d[:])

            x_lo = xd_d[0:SPLIT, :]
            x_hi = xd_d[SPLIT:N_PAD, :]

            qrr = [0]

            def emit_gather(b, nch_s, idx_tile, src_ap, tag):
                ch = min(BATCH_CH, nch_s - b * BATCH_CH)
                ni = ch * P
                mt = msgs_pool.tile([P, BATCH_CH * 2 * D], mybir.dt.bfloat16, tag=tag)
                nc.gpsimd.dma_gather(
                    out_ap=mt[:, :ch * 2 * D].rearrange("p (c e) -> p c e", e=2 * D),
                    in_ap=src_ap,
                    idxs_ap=idx_tile[:, b * BATCH_CH * 8:b * BATCH_CH * 8 + ch * 8],
                    num_idxs=ni, num_idxs_reg=ni, elem_size=2 * D,
                    single_packet=False, queue_num=qrr[0] % 4)
                qrr[0] += 1
                return mt

            # emit gathers interleaved in bucket-consumption order
            nb_lo = (nch_lo + BATCH_CH - 1) // BATCH_CH
            nb_hi = (nch_hi + BATCH_CH - 1) // BATCH_CH
            ev = []
            for b in range(nb_lo):
                ev.append((b * BATCH_CH // c_lo, 0, b))
            for b in range(nb_hi):
                ev.append((b * BATCH_CH // c_hi, 1, b))
            ev.sort()
            lo_tiles, hi_tiles = {}, {}
            for _, s, b in ev:
                if s == 0:
                    lo_tiles[b] = emit_gather(b, nch_lo, il_t, x_lo, "mlo")
                else:
                    hi_tiles[b] = emit_gather(b, nch_hi, ih_t, x_hi, "mhi")

            stg = None
            for w in range(WPC):
                o2 = out2_pool.tile([D, P], mybir.dt.float32, space="PSUM", tag="o2")
                for r in range(R):
                    bk = w * R + r
                    ps = agg_pool.tile([D, P], mybir.dt.float32, space="PSUM", tag="agg")
                    for c in range(cpw):
                        j = bk * cpw + c
                        if c < c_lo:
                            js = bk * c_lo + c
                            mt = lo_tiles[js // BATCH_CH]
                        else:
                            js = bk * c_hi + (c - c_lo)
                            mt = hi_tiles[js // BATCH_CH]
                        jl = js % BATCH_CH
                        oh = oh_pool.tile([P, P], mybir.dt.bfloat16, tag="oh")
                        nc.vector.tensor_scalar(
                            out=oh[:], in0=iota_t[:],
                            scalar1=tgt_t[:, j:j + 1], scalar2=wgt_t[:, j:j + 1],
                            op0=mybir.AluOpType.is_equal, op1=mybir.AluOpType.mult)
                        nc.tensor.matmul(
                            out=ps[:], lhsT=mt[:, jl * 2 * D:jl * 2 * D + D],
                            rhs=oh[:], start=(c == 0), stop=(c == cpw - 1))
                    ab = absb_pool.tile([D, P], mybir.dt.bfloat16, tag="ab")
                    nc.scalar.copy(ab[:], ps[:])
                    nc.tensor.matmul(
                        out=o2[:], lhsT=wtab_t[:, r * D:(r + 1) * D], rhs=ab[:],
                        start=(r == 0), stop=(r == R - 1))
                si = w % 8
                if si == 0:
                    stg = evict_pool.tile([D, 8 * P], mybir.dt.float32, tag="stg")
                nc.scalar.copy(stg[:, si * P:(si + 1) * P], o2[:])
                if si == 7 or w == WPC - 1:
                    w0 = w - si
                    nc.sync.dma_start(
                        out_d[:, w0 * P:(w + 1) * P], stg[:, :(si + 1) * P])

    nc.compile()
    return nc


def _prep_core(src, tgt_off, wgt, c_lo, c_hi):
    """Build per-core edge streams. src = gather row index (int64),
    tgt_off = (bucket index, target offset within window), already sorted by
    bucket. Returns il, ih, wgt_arr, tgt_arr."""
    cpw = c_lo + c_hi
    nb = NWIN_PC
    nch_lo = nb * c_lo
    nch_hi = nb * c_hi
    nch = nb * cpw

    bidx, toff = tgt_off

    il = np.zeros(nch_lo * P, np.int16)
    ih = np.zeros(nch_hi * P, np.int16)
    wgt_arr = np.zeros((P, nch), np.float32)
    tgt_arr = np.zeros((P, nch), np.float32)

    lo_mask = src < SPLIT
    starts = np.searchsorted(bidx, np.arange(nb + 1))
    for b in range(nb):
        s0, s1 = starts[b], starts[b + 1]
        lm = lo_mask[s0:s1]
        for is_lo, c_n, stream, base in ((True, c_lo, il, 0), (False, c_hi, ih, SPLIT)):
            sel = lm if is_lo else ~lm
            n = int(sel.sum())
            cap = c_n * P
            assert n <= cap, (b, n, cap)
            pos0 = b * cap
            stream[pos0:pos0 + n] = (src[s0:s1][sel] - base).astype(np.int16)
            # slot -> global chunk j and lane
            slots = np.arange(cap)
            cw = slots // P
            lane = slots % P
            j = b * cpw + (cw if is_lo else c_lo + cw)
            wcol = np.zeros(cap, np.float32)
            tcol = np.zeros(cap, np.float32)
            wcol[:n] = wgt[s0:s1][sel]
            tcol[:n] = toff[s0:s1][sel]
            wgt_arr[lane, j] = wcol
            tgt_arr[lane, j] = tcol

    def wrap(stream, nch_s):
        # slot (chunk js, lane p) at stream[js*128+p] -> wrapped (128, nch_s*8)
        out = np.zeros((P, nch_s * 8), np.int16)
        nbt = (nch_s + BATCH_CH - 1) // BATCH_CH
        for b in range(nbt):
            ch = min(BATCH_CH, nch_s - b * BATCH_CH)
            seg = stream[b * BATCH_CH * P: b * BATCH_CH * P + ch * P]
            w16 = seg.reshape(ch * 8, 16).T           # (16, ch*8)
            out[:, b * BATCH_CH * 8: b * BATCH_CH * 8 + ch * 8] = np.tile(w16, (8, 1))
        return out

    return wrap(il, nch_lo), wrap(ih, nch_hi), wgt_arr, tgt_arr


def _bf16(a):
    import ml_dtypes
    return a.astype(ml_dtypes.bfloat16)


def kernel(x, blocks, edge_weights, source, target, edge_type):
    from concourse.bass_utils import run_bass_kernel_spmd

    x = np.asarray(x, np.float32)
    blocks = np.asarray(blocks, np.float32)
    edge_weights = np.asarray(edge_weights, np.float32)
    source = np.asarray(source, np.int64)
    target = np.asarray(target, np.int64)
    edge_type = np.asarray(edge_type, np.int64)

    n, d = x.shape
    assert n == N_NODES and d == D

    xp = np.zeros((N_PAD, 2 * D), np.float32)
    xp[:n, :D] = x
    xp[:n, D:] = x
    xd = _bf16(xp)

    iota = _bf16(np.broadcast_to(np.arange(P, dtype=np.float32), (P, P)).copy())

    # W table: same for all cores (blocks replicated)
    bs = D // blocks.shape[1]
    wtab = np.zeros((D, R * D), np.float32)
    for r in range(R):
        for b in range(blocks.shape[1]):
            wtab[b * bs:(b + 1) * bs, r * D + b * bs:r * D + (b + 1) * bs] = blocks[r, b]
    wtab = _bf16(wtab)

    # per-core edge sets (sorted by bucket) + stream capacities
    win = target // P
    core = np.minimum(win // WPC, R - 1)
    per_core = []
    c_lo = c_hi = 1
    for c in range(R):
        m = core == c
        src_c = source[m]
        bidx = (win[m] - c * WPC) * R + edge_type[m]     # bucket index
        toff = (target[m] % P).astype(np.float32)
        wgt_c = edge_weights[m]
        order = np.argsort(bidx, kind="stable")
        src_c, bidx, toff, wgt_c = (a[order] for a in (src_c, bidx, toff, wgt_c))
        lo = src_c < SPLIT
        cnt_lo = np.bincount(bidx[lo], minlength=NWIN_PC)
        cnt_hi = np.bincount(bidx[~lo], minlength=NWIN_PC)
        c_lo = max(c_lo, int(-(-cnt_lo.max() // P)))
        c_hi = max(c_hi, int(-(-cnt_hi.max() // P)))
        per_core.append((src_c, bidx, toff, wgt_c))

    key = (c_lo, c_hi)
    if key not in _cache:
        _cache[key] = _build_program(c_lo, c_hi)
    nc = _cache[key]

    in_maps = []
    for c in range(R):
        src_c, bidx, toff, wgt_c = per_core[c]
        il, ih, wgt_arr, tgt_arr = _prep_core(src_c, (bidx, toff), wgt_c, c_lo, c_hi)
        in_maps.append({
            "xd": xd, "il": il, "ih": ih,
            "wgt": wgt_arr, "tgt": tgt_arr,
            "iota": iota, "wtab": wtab,
        })

    res = run_bass_kernel_spmd(nc, in_maps, core_ids=list(range(R)))

    out = np.zeros((R * SLAB, D), np.float32)
    for c in range(R):
        outT = res.results[c]["outT"]                    # [64, 6272]
        out[c * SLAB:(c + 1) * SLAB] = outT.T
    return out[:N_NODES]
